# revision 13
# baseline (speedup 1.0000x reference)
"""Distributed Trainium2 kernel for nn_Attention_14697378086932.

Head-sharded (tensor-parallel) multi-head attention over 8 NeuronCores:
each core computes 2 of the 16 heads end-to-end.

Per core c:
  - QKV projections:  Q^T = Wq_c^T @ X^T  (f32r matmuls, contraction on
    hidden dim), giving Q^T/K^T/V^T in [128 local channels, 4096 tokens]
    layout (d-on-partitions), which is exactly the layout the scores
    matmul wants.
  - rotary: only global channels 0..63 are rotated (reference quirk), i.e.
    local channels 0..63 of core 0.  All cores run the same graph; cores
    1..7 receive cos=1/sin=0 so their "rotation" is the identity.
    rotate_half is a fixed 64x64 permutation matrix applied on the PE.
  - attention (per batch, per local head, flash-style over 128-token key
    chunks): S^T = K Q^T (f32r), P^T = exp(S^T) on the scalar engine
    (no max subtraction -- logits are bounded, f32/bf16 exp is safe),
    O^T = [V | 1]^T @ P^T (bf16) which yields the softmax denominator as
    a free 65th row.  Normalize with a reciprocal multiply.
  - output projection: partial = O_loc @ Wo_c (bf16), DMA'd out per
    128-token chunk.
Host sums the 8 partial outputs and adds bo.
"""
import sys
import types

sys.path.insert(0, "/opt/trn_rl_repo")

import numpy as np
import ml_dtypes

import concourse.bass as bass
import concourse.mybir as mybir
from concourse import bacc
from concourse.bass import ts, ds
from concourse.tile import TileContext
from concourse.masks import make_identity
from concourse.bass_utils import run_bass_kernel_spmd

F32 = mybir.dt.float32
F32R = mybir.dt.float32r
BF16 = mybir.dt.bfloat16

P = 128          # partitions / local channels per core
HID = 1024       # hidden
NT = 4096        # total tokens (batch 2 x 2048)
NB = 2048        # tokens per batch
HD = 64          # head dim
N_CORES = 8

_NC_CACHE = None


def build_nc():
    nc = bacc.Bacc("TRN2")

    xt = nc.declare_dram_parameter("xt", [HID, NT], F32R, isOutput=False)
    wq = nc.declare_dram_parameter("wq", [HID, P], F32R, isOutput=False)
    wk = nc.declare_dram_parameter("wk", [HID, P], F32R, isOutput=False)
    wv = nc.declare_dram_parameter("wv", [HID, P], F32R, isOutput=False)
    wo = nc.declare_dram_parameter("wo", [P, HID], BF16, isOutput=False)
    bia = nc.declare_dram_parameter("bias", [P, 3], F32, isOutput=False)
    cos = nc.declare_dram_parameter("cos", [HD, NT], F32, isOutput=False)
    sin = nc.declare_dram_parameter("sin", [HD, NT], F32, isOutput=False)
    rmat = nc.declare_dram_parameter("rmat", [HD, HD], F32R, isOutput=False)
    out = nc.declare_dram_parameter("out", [NT, HID], F32, isOutput=True)

    xt_r = xt[:].rearrange("(o p) n -> p o n", p=P)      # [128, 8, 4096]
    wq_r = wq[:].rearrange("(o p) m -> p o m", p=P)      # [128, 8, 128]
    wk_r = wk[:].rearrange("(o p) m -> p o m", p=P)
    wv_r = wv[:].rearrange("(o p) m -> p o m", p=P)

    with TileContext(nc) as tc:
        with tc.tile_pool(name="consts", bufs=1) as consts, \
             tc.tile_pool(name="big", bufs=1) as big:
            wqs = consts.tile([P, 8, P], F32R)
            wks = consts.tile([P, 8, P], F32R)
            wvs = consts.tile([P, 8, P], F32R)
            nc.sync.dma_start(wqs, wq_r)
            nc.sync.dma_start(wks, wk_r)
            nc.sync.dma_start(wvs, wv_r)
            wos = consts.tile([P, HID], BF16)
            nc.sync.dma_start(wos, wo[:])
            bias_t = consts.tile([P, 3], F32)
            nc.sync.dma_start(bias_t, bia[:])
            cos_t = consts.tile([HD, NT], F32)
            sin_t = consts.tile([HD, NT], F32)
            nc.sync.dma_start(cos_t, cos[:])
            nc.sync.dma_start(sin_t, sin[:])
            rmat_t = consts.tile([HD, HD], F32R)
            nc.sync.dma_start(rmat_t, rmat[:])
            ident = consts.tile([P, P], F32)
            make_identity(nc, ident)

            Qt = big.tile([P, NT], F32R)     # Q^T (local channels x tokens)
            Kt = big.tile([P, NT], F32R)
            Vt = big.tile([P, NT], F32)      # V^T, pre-transpose
            Ot = big.tile([P, NT], BF16)     # normalized attention out^T
            # V in natural [token, channel] layout + ones column, per head,
            # per 128-token key chunk: [128 tok, 32 chunks, 64 V | 1 | pad]
            VaugA = big.tile([P, 32, 66], BF16)
            VaugB = big.tile([P, 32, 66], BF16)
            nc.vector.memset(VaugA[:, :, 64:66], 1.0)
            nc.vector.memset(VaugB[:, :, 64:66], 1.0)

            # ---------------- Phase A: QKV projections + rope + V transpose
            with tc.tile_pool(name="xtp", bufs=2) as xtp, \
                 tc.tile_pool(name="ropet", bufs=2) as ropet, \
                 tc.tile_pool(name="psA", bufs=4, space="PSUM") as psA:
                for cc in range(8):   # 512-token chunks
                    sl = ts(cc, 512)
                    xtt = xtp.tile([P, 8, 512], F32R, tag="xt")
                    nc.sync.dma_start(xtt, xt_r[:, :, sl])
                    for wt, bidx, dst in ((wqs, 0, Qt), (wks, 1, Kt), (wvs, 2, Vt)):
                        ps = psA.tile([P, 512], F32, tag="ps")
                        for o in range(8):
                            nc.tensor.matmul(ps, wt[:, o], xtt[:, o],
                                             start=(o == 0), stop=(o == 7))
                        nc.scalar.activation(
                            dst[:, sl], ps,
                            mybir.ActivationFunctionType.Identity,
                            bias=bias_t[:, bidx:bidx + 1])
                    # rope on first 64 local channels of Q and K
                    for t in (Qt, Kt):
                        psr = psA.tile([P, 512], F32, tag="ps")
                        nc.tensor.matmul(psr[0:HD], rmat_t,
                                         t[0:HD, sl],
                                         start=True, stop=True)
                        tmp = ropet.tile([HD, 512], F32, tag="tmp")
                        nc.vector.tensor_tensor(tmp, psr[0:HD], sin_t[:, sl],
                                                mybir.AluOpType.mult)
                        nc.vector.tensor_tensor(t[0:HD, sl], t[0:HD, sl],
                                                cos_t[:, sl], mybir.AluOpType.mult)
                        nc.vector.tensor_tensor(t[0:HD, sl], t[0:HD, sl], tmp,
                                                mybir.AluOpType.add)
                    # V transpose into per-head natural layout (+ones col)
                    for s in range(4):
                        kc = cc * 4 + s
                        pst = psA.tile([P, 512], F32, tag="ps")
                        nc.tensor.transpose(pst[:, 0:P], Vt[:, ts(kc, P)], ident)
                        nc.vector.tensor_copy(VaugA[:, kc, 0:HD], pst[:, 0:HD])
                        nc.vector.tensor_copy(VaugB[:, kc, 0:HD], pst[:, HD:P])

            # ---------------- Phase B: attention + output projection
            # Both local heads are processed together per (batch, nq-block):
            # their S^T matmuls contract only 64 partitions each, so head A
            # (rows 0..63) and head B (rows 64..127) run CONCURRENTLY in
            # different PE row-groups (tile_position auto-derived from the
            # APs' base partitions).  The exp stream on ScalarE is the
            # bottleneck; PE has enough slack that the output projection of
            # the previous block can borrow the freed O-psum slots.
            with tc.tile_pool(name="ptp", bufs=4) as ptp, \
                 tc.tile_pool(name="osb", bufs=3) as osb, \
                 tc.tile_pool(name="nrm", bufs=2) as nrm, \
                 tc.tile_pool(name="spSA", bufs=1, space="PSUM") as spSA, \
                 tc.tile_pool(name="spSB", bufs=1, space="PSUM") as spSB, \
                 tc.tile_pool(name="spOA", bufs=1, space="PSUM") as spOA, \
                 tc.tile_pool(name="spOB", bufs=1, space="PSUM") as spOB:

                def oproj(q0):
                    # output projection for 1024 tokens (both heads), psum
                    # borrowed from the O slots the normalize just released
                    for tch in range(8):
                        t0 = q0 + tch * P
                        pool = spOA if tch % 2 == 0 else spOB
                        Pps = pool.tile([P, 1024], F32, tag="O")
                        for hf in range(2):
                            nc.tensor.matmul(
                                Pps[:, ts(hf, 512)],
                                Ot[:, t0:t0 + P],
                                wos[:, ts(hf, 512)],
                                start=True, stop=True)
                        ost = osb.tile([P, HID], F32, tag="ost")
                        nc.any.tensor_copy(ost, Pps)
                        nc.sync.dma_start(out[t0:t0 + P, :], ost)

                prev_q0 = None
                for b in range(2):
                    for nqb in range(2):
                        q0 = b * NB + nqb * 1024

                        def s_exp(i, hlo, spool, ptag):
                            k0 = b * NB + i * P
                            Sps = spool.tile([P, 1024], F32, tag="S")
                            for hf in range(2):
                                nc.tensor.matmul(
                                    Sps[:, ts(hf, 512)],
                                    Kt[hlo:hlo + HD, k0:k0 + P],
                                    Qt[hlo:hlo + HD, ds(q0 + hf * 512, 512)],
                                    start=True, stop=True)
                            Pt = ptp.tile([P, 1024], BF16, tag=ptag)
                            nc.scalar.activation(
                                Pt, Sps, mybir.ActivationFunctionType.Exp)
                            return Pt

                        def pv(i, Pt, Vaug, Ops):
                            kc = b * 16 + i
                            for hf in range(2):
                                nc.tensor.matmul(
                                    Ops[:, ts(hf, 512)],
                                    Vaug[:, kc, 0:HD + 1],
                                    Pt[:, ts(hf, 512)],
                                    start=(i == 0), stop=(i == 15),
                                    skip_group_check=True)

                        # software pipeline: S^T/exp one chunk ahead of PV
                        pa = s_exp(0, 0, spSA, "PA")
                        pb = s_exp(0, HD, spSB, "PB")
                        if prev_q0 is not None:
                            oproj(prev_q0)
                        # allocate AFTER oproj(prev) so the O-slot reuse
                        # chain is Ops(j) -> oproj(j) -> Ops(j+1)
                        OpsA = spOA.tile([HD + 1, 1024], F32, tag="O")
                        OpsB = spOB.tile([HD + 1, 1024], F32, tag="O")
                        for i in range(1, 16):
                            na = s_exp(i, 0, spSA, "PA")
                            nb = s_exp(i, HD, spSB, "PB")
                            pv(i - 1, pa, VaugA, OpsA)
                            pv(i - 1, pb, VaugB, OpsB)
                            pa, pb = na, nb
                        pv(15, pa, VaugA, OpsA)
                        pv(15, pb, VaugB, OpsB)

                        # normalize: copy out of PSUM right away so the O
                        # banks free up, then rows 0..63 / row 64 from SBUF
                        for hlo, Ops in ((0, OpsA), (HD, OpsB)):
                            osum = nrm.tile([HD + 1, 1024], F32, tag="osum")
                            nc.vector.tensor_copy(osum, Ops)
                            rc = nrm.tile([1, 1024], F32, tag="rc")
                            nc.vector.reciprocal(rc, osum[HD:HD + 1, :])
                            rcb = nrm.tile([HD, 1024], F32, tag="rcb")
                            nc.gpsimd.partition_broadcast(rcb, rc)
                            nc.vector.tensor_tensor(
                                Ot[hlo:hlo + HD, ds(q0, 1024)],
                                osum[0:HD, :],
                                rcb,
                                mybir.AluOpType.mult)
                        prev_q0 = q0
                oproj(prev_q0)

    nc.compile()
    return nc


def _get_nc():
    global _NC_CACHE
    if _NC_CACHE is None:
        _NC_CACHE = build_nc()
    return _NC_CACHE


def shard_inputs(x, rope_cos, rope_sin, Wq, bq, Wk, bk, Wv, bv, Wo, bo):
    """Build per-core input maps."""
    xt = np.ascontiguousarray(x.reshape(NT, HID).T).astype(np.float32)
    cosT = np.ascontiguousarray(rope_cos.reshape(NT, HD).T).astype(np.float32)
    sinT = np.ascontiguousarray(rope_sin.reshape(NT, HD).T).astype(np.float32)
    cos_id = np.ones((HD, NT), np.float32)
    sin_id = np.zeros((HD, NT), np.float32)
    # rotate_half as matrix R: out = R @ t, R[2i,2i+1]=-1, R[2i+1,2i]=+1.
    # matmul computes lhsT.T @ rhs, so pass R.T.
    R = np.zeros((HD, HD), np.float32)
    idx = np.arange(0, HD, 2)
    R[idx, idx + 1] = -1.0
    R[idx + 1, idx] = 1.0
    rmat = np.ascontiguousarray(R.T)

    in_maps = []
    for c in range(N_CORES):
        lo, hi = c * P, (c + 1) * P
        in_maps.append({
            "xt": xt,
            "wq": np.ascontiguousarray(Wq[:, lo:hi]).astype(np.float32),
            "wk": np.ascontiguousarray(Wk[:, lo:hi]).astype(np.float32),
            "wv": np.ascontiguousarray(Wv[:, lo:hi]).astype(np.float32),
            "wo": np.ascontiguousarray(Wo[lo:hi, :]).astype(ml_dtypes.bfloat16),
            "bias": np.ascontiguousarray(
                np.stack([bq[lo:hi], bk[lo:hi], bv[lo:hi]], axis=1)
            ).astype(np.float32),
            "cos": cosT if c == 0 else cos_id,
            "sin": sinT if c == 0 else sin_id,
            "rmat": rmat,
        })
    return in_maps


def run_device(inputs, trace=False, **kw):
    nc = _get_nc()
    in_maps = shard_inputs(**inputs)
    res = run_bass_kernel_spmd(nc, in_maps, core_ids=list(range(N_CORES)),
                               trace=trace, **kw)
    return res


def gather(res, bo):
    acc = res.results[0]["out"].astype(np.float32).copy()
    for c in range(1, N_CORES):
        acc += res.results[c]["out"]
    acc += bo[None, :].astype(np.float32)
    return acc.reshape(2, NB, HID)


def kernel(**inputs):
    res = run_device(inputs, trace=False)
    return gather(res, np.asarray(inputs["bo"], np.float32))


# revision 18
# speedup vs baseline: 1.0161x; 1.0161x over previous
"""Distributed Trainium2 kernel for nn_Attention_14697378086932.

Head-sharded (tensor-parallel) multi-head attention over 8 NeuronCores:
each core computes 2 of the 16 heads end-to-end.

Per core c:
  - QKV projections:  Q^T = Wq_c^T @ X^T  (f32r matmuls, contraction on
    hidden dim), giving Q^T/K^T/V^T in [128 local channels, 4096 tokens]
    layout (d-on-partitions), which is exactly the layout the scores
    matmul wants.
  - rotary: only global channels 0..63 are rotated (reference quirk), i.e.
    local channels 0..63 of core 0.  All cores run the same graph; cores
    1..7 receive cos=1/sin=0 so their "rotation" is the identity.
    rotate_half is a fixed 64x64 permutation matrix applied on the PE.
  - attention (per batch, per local head, flash-style over 128-token key
    chunks): S^T = K Q^T (f32r), P^T = exp(S^T) on the scalar engine
    (no max subtraction -- logits are bounded, f32/bf16 exp is safe),
    O^T = [V | 1]^T @ P^T (bf16) which yields the softmax denominator as
    a free 65th row.  Normalize with a reciprocal multiply.
  - output projection: partial = O_loc @ Wo_c (bf16), DMA'd out per
    128-token chunk.
Host sums the 8 partial outputs and adds bo.
"""
import sys
import types

sys.path.insert(0, "/opt/trn_rl_repo")

import numpy as np
import ml_dtypes

import concourse.bass as bass
import concourse.mybir as mybir
from concourse import bacc
from concourse.bass import ts, ds
from concourse.tile import TileContext
from concourse.masks import make_identity
from concourse.bass_utils import run_bass_kernel_spmd

F32 = mybir.dt.float32
F32R = mybir.dt.float32r
BF16 = mybir.dt.bfloat16

P = 128          # partitions / local channels per core
HID = 1024       # hidden
NT = 4096        # total tokens (batch 2 x 2048)
NB = 2048        # tokens per batch
HD = 64          # head dim
N_CORES = 8

_NC_CACHE = None


def build_nc():
    nc = bacc.Bacc("TRN2")

    xt = nc.declare_dram_parameter("xt", [HID, NT], F32R, isOutput=False)
    wq = nc.declare_dram_parameter("wq", [HID, P], F32R, isOutput=False)
    wk = nc.declare_dram_parameter("wk", [HID, P], F32R, isOutput=False)
    wv = nc.declare_dram_parameter("wv", [HID, P], F32R, isOutput=False)
    wo = nc.declare_dram_parameter("wo", [P, HID], BF16, isOutput=False)
    bia = nc.declare_dram_parameter("bias", [P, 3], F32, isOutput=False)
    cos = nc.declare_dram_parameter("cos", [HD, NT], F32, isOutput=False)
    sin = nc.declare_dram_parameter("sin", [HD, NT], F32, isOutput=False)
    rmat = nc.declare_dram_parameter("rmat", [HD, HD], F32R, isOutput=False)
    out = nc.declare_dram_parameter("out", [NT, HID], F32, isOutput=True)

    xt_r = xt[:].rearrange("(o p) n -> p o n", p=P)      # [128, 8, 4096]
    wq_r = wq[:].rearrange("(o p) m -> p o m", p=P)      # [128, 8, 128]
    wk_r = wk[:].rearrange("(o p) m -> p o m", p=P)
    wv_r = wv[:].rearrange("(o p) m -> p o m", p=P)

    with TileContext(nc) as tc:
        with tc.tile_pool(name="consts", bufs=1) as consts, \
             tc.tile_pool(name="big", bufs=1) as big:
            wqs = consts.tile([P, 8, P], F32R)
            wks = consts.tile([P, 8, P], F32R)
            wvs = consts.tile([P, 8, P], F32R)
            nc.sync.dma_start(wqs, wq_r)
            nc.sync.dma_start(wks, wk_r)
            nc.sync.dma_start(wvs, wv_r)
            wos = consts.tile([P, HID], BF16)
            nc.sync.dma_start(wos, wo[:])
            bias_t = consts.tile([P, 3], F32)
            nc.sync.dma_start(bias_t, bia[:])
            rmat_t = consts.tile([HD, HD], F32R)
            nc.sync.dma_start(rmat_t, rmat[:])
            ident = consts.tile([P, P], F32)
            make_identity(nc, ident)

            Qt = big.tile([P, NT], F32R)     # Q^T (local channels x tokens)
            Kt = big.tile([P, NT], F32R)
            Vt = big.tile([P, NT], F32)      # V^T, pre-transpose
            Ot = big.tile([P, NT], BF16)     # normalized attention out^T
            # V in natural [token, channel] layout + ones column, per head,
            # per 128-token key chunk: [128 tok, 32 chunks, 64 V | 1 | pad]
            VaugA = big.tile([P, 32, 66], BF16)
            VaugB = big.tile([P, 32, 66], BF16)
            nc.vector.memset(VaugA[:, :, 64:66], 1.0)
            nc.vector.memset(VaugB[:, :, 64:66], 1.0)

            # ---------------- Phase A: QKV projections + rope + V transpose
            with tc.tile_pool(name="xtp", bufs=3) as xtp, \
                 tc.tile_pool(name="ropet", bufs=2) as ropet, \
                 tc.tile_pool(name="trig", bufs=1) as trig, \
                 tc.tile_pool(name="psA", bufs=4, space="PSUM") as psA:
                cos_t = trig.tile([HD, NT], F32)
                sin_t = trig.tile([HD, NT], F32)
                nc.sync.dma_start(cos_t, cos[:])
                nc.sync.dma_start(sin_t, sin[:])
                for cc in range(8):   # 512-token chunks
                    sl = ts(cc, 512)
                    xtt = xtp.tile([P, 8, 512], F32R, tag="xt")
                    nc.sync.dma_start(xtt, xt_r[:, :, sl])
                    for wt, bidx, dst in ((wqs, 0, Qt), (wks, 1, Kt), (wvs, 2, Vt)):
                        ps = psA.tile([P, 512], F32, tag="ps")
                        for o in range(8):
                            nc.tensor.matmul(ps, wt[:, o], xtt[:, o],
                                             start=(o == 0), stop=(o == 7))
                        nc.scalar.activation(
                            dst[:, sl], ps,
                            mybir.ActivationFunctionType.Identity,
                            bias=bias_t[:, bidx:bidx + 1])
                    # rope on first 64 local channels of Q and K
                    for t in (Qt, Kt):
                        psr = psA.tile([P, 512], F32, tag="ps")
                        nc.tensor.matmul(psr[0:HD], rmat_t,
                                         t[0:HD, sl],
                                         start=True, stop=True)
                        tmp = ropet.tile([HD, 512], F32, tag="tmp")
                        nc.vector.tensor_tensor(tmp, psr[0:HD], sin_t[:, sl],
                                                mybir.AluOpType.mult)
                        nc.vector.tensor_tensor(t[0:HD, sl], t[0:HD, sl],
                                                cos_t[:, sl], mybir.AluOpType.mult)
                        nc.vector.tensor_tensor(t[0:HD, sl], t[0:HD, sl], tmp,
                                                mybir.AluOpType.add)
                    # V transpose into per-head natural layout (+ones col)
                    for s in range(4):
                        kc = cc * 4 + s
                        pst = psA.tile([P, 512], F32, tag="ps")
                        nc.tensor.transpose(pst[:, 0:P], Vt[:, ts(kc, P)], ident)
                        nc.vector.tensor_copy(VaugA[:, kc, 0:HD], pst[:, 0:HD])
                        nc.vector.tensor_copy(VaugB[:, kc, 0:HD], pst[:, HD:P])

            # ---------------- Phase B: attention + output projection
            # Both local heads are processed together per (batch, nq-block):
            # their S^T matmuls contract only 64 partitions each, so head A
            # (rows 0..63) and head B (rows 64..127) run CONCURRENTLY in
            # different PE row-groups (tile_position auto-derived from the
            # APs' base partitions).  The exp stream on ScalarE is the
            # bottleneck; PE has enough slack that the output projection of
            # the previous block can borrow the freed O-psum slots.
            with tc.tile_pool(name="ptp", bufs=7) as ptp, \
                 tc.tile_pool(name="osb", bufs=3) as osb, \
                 tc.tile_pool(name="nrm", bufs=2) as nrm, \
                 tc.tile_pool(name="spSA", bufs=1, space="PSUM") as spSA, \
                 tc.tile_pool(name="spSB", bufs=1, space="PSUM") as spSB, \
                 tc.tile_pool(name="spOA", bufs=1, space="PSUM") as spOA, \
                 tc.tile_pool(name="spOB", bufs=1, space="PSUM") as spOB:

                def oproj(q0):
                    # output projection for 1024 tokens (both heads), psum
                    # borrowed from the O slots the normalize just released
                    for tch in range(8):
                        t0 = q0 + tch * P
                        pool = spOA if tch % 2 == 0 else spOB
                        Pps = pool.tile([P, 1024], F32, tag="O")
                        for hf in range(2):
                            nc.tensor.matmul(
                                Pps[:, ts(hf, 512)],
                                Ot[:, t0:t0 + P],
                                wos[:, ts(hf, 512)],
                                start=True, stop=True)
                        ost = osb.tile([P, HID], F32, tag="ost")
                        nc.any.tensor_copy(ost, Pps)
                        nc.sync.dma_start(out[t0:t0 + P, :], ost)

                prev_q0 = None
                for b in range(2):
                    for nqb in range(2):
                        q0 = b * NB + nqb * 1024

                        def s_exp(i, hlo, spool, ptag):
                            k0 = b * NB + i * P
                            Sps = spool.tile([P, 1024], F32, tag="S")
                            for hf in range(2):
                                nc.tensor.matmul(
                                    Sps[:, ts(hf, 512)],
                                    Kt[hlo:hlo + HD, k0:k0 + P],
                                    Qt[hlo:hlo + HD, ds(q0 + hf * 512, 512)],
                                    start=True, stop=True)
                            Pt = ptp.tile([P, 1024], BF16, tag=ptag)
                            nc.scalar.activation(
                                Pt, Sps, mybir.ActivationFunctionType.Exp)
                            return Pt

                        def pv(i, Pt, Vaug, Ops):
                            kc = b * 16 + i
                            for hf in range(2):
                                nc.tensor.matmul(
                                    Ops[:, ts(hf, 512)],
                                    Vaug[:, kc, 0:HD + 1],
                                    Pt[:, ts(hf, 512)],
                                    start=(i == 0), stop=(i == 15),
                                    skip_group_check=True)

                        # software pipeline: S^T/exp run DEPTH chunks ahead
                        # of PV, so the PE never queues behind the exp
                        # stream and the previous block's output projection
                        # (which must wait for that block's normalize) gets
                        # enough runway to never stall the PE FIFO.
                        DEPTH = 5
                        pend = []
                        for i in range(DEPTH):
                            pend.append((s_exp(i, 0, spSA, "PA"),
                                         s_exp(i, HD, spSB, "PB")))
                        if prev_q0 is not None:
                            oproj(prev_q0)
                        # allocate AFTER oproj(prev) so the O-slot reuse
                        # chain is Ops(j) -> oproj(j) -> Ops(j+1)
                        OpsA = spOA.tile([HD + 1, 1024], F32, tag="O")
                        OpsB = spOB.tile([HD + 1, 1024], F32, tag="O")
                        for i in range(DEPTH, 16):
                            na = s_exp(i, 0, spSA, "PA")
                            nb = s_exp(i, HD, spSB, "PB")
                            pa, pb = pend.pop(0)
                            pv(i - DEPTH, pa, VaugA, OpsA)
                            pv(i - DEPTH, pb, VaugB, OpsB)
                            pend.append((na, nb))
                        for j, (pa, pb) in enumerate(pend):
                            pv(16 - DEPTH + j, pa, VaugA, OpsA)
                            pv(16 - DEPTH + j, pb, VaugB, OpsB)

                        # normalize: copy out of PSUM right away so the O
                        # banks free up, then rows 0..63 / row 64 from SBUF
                        for hlo, Ops in ((0, OpsA), (HD, OpsB)):
                            osum = nrm.tile([HD + 1, 1024], F32, tag="osum")
                            nc.vector.tensor_copy(osum, Ops)
                            rc = nrm.tile([1, 1024], F32, tag="rc")
                            nc.vector.reciprocal(rc, osum[HD:HD + 1, :])
                            rcb = nrm.tile([HD, 1024], F32, tag="rcb")
                            nc.gpsimd.partition_broadcast(rcb, rc)
                            nc.vector.tensor_tensor(
                                Ot[hlo:hlo + HD, ds(q0, 1024)],
                                osum[0:HD, :],
                                rcb,
                                mybir.AluOpType.mult)
                        prev_q0 = q0
                oproj(prev_q0)

    nc.compile()
    return nc


def _get_nc():
    global _NC_CACHE
    if _NC_CACHE is None:
        _NC_CACHE = build_nc()
    return _NC_CACHE


def shard_inputs(x, rope_cos, rope_sin, Wq, bq, Wk, bk, Wv, bv, Wo, bo):
    """Build per-core input maps."""
    xt = np.ascontiguousarray(x.reshape(NT, HID).T).astype(np.float32)
    cosT = np.ascontiguousarray(rope_cos.reshape(NT, HD).T).astype(np.float32)
    sinT = np.ascontiguousarray(rope_sin.reshape(NT, HD).T).astype(np.float32)
    cos_id = np.ones((HD, NT), np.float32)
    sin_id = np.zeros((HD, NT), np.float32)
    # rotate_half as matrix R: out = R @ t, R[2i,2i+1]=-1, R[2i+1,2i]=+1.
    # matmul computes lhsT.T @ rhs, so pass R.T.
    R = np.zeros((HD, HD), np.float32)
    idx = np.arange(0, HD, 2)
    R[idx, idx + 1] = -1.0
    R[idx + 1, idx] = 1.0
    rmat = np.ascontiguousarray(R.T)

    in_maps = []
    for c in range(N_CORES):
        lo, hi = c * P, (c + 1) * P
        in_maps.append({
            "xt": xt,
            "wq": np.ascontiguousarray(Wq[:, lo:hi]).astype(np.float32),
            "wk": np.ascontiguousarray(Wk[:, lo:hi]).astype(np.float32),
            "wv": np.ascontiguousarray(Wv[:, lo:hi]).astype(np.float32),
            "wo": np.ascontiguousarray(Wo[lo:hi, :]).astype(ml_dtypes.bfloat16),
            "bias": np.ascontiguousarray(
                np.stack([bq[lo:hi], bk[lo:hi], bv[lo:hi]], axis=1)
            ).astype(np.float32),
            "cos": cosT if c == 0 else cos_id,
            "sin": sinT if c == 0 else sin_id,
            "rmat": rmat,
        })
    return in_maps


def run_device(inputs, trace=False, **kw):
    nc = _get_nc()
    in_maps = shard_inputs(**inputs)
    res = run_bass_kernel_spmd(nc, in_maps, core_ids=list(range(N_CORES)),
                               trace=trace, **kw)
    return res


def gather(res, bo):
    acc = res.results[0]["out"].astype(np.float32).copy()
    for c in range(1, N_CORES):
        acc += res.results[c]["out"]
    acc += bo[None, :].astype(np.float32)
    return acc.reshape(2, NB, HID)


def kernel(**inputs):
    res = run_device(inputs, trace=False)
    return gather(res, np.asarray(inputs["bo"], np.float32))
